# revision 55
# baseline (speedup 1.0000x reference)
"""Trainium2 Bass kernel for nn_DecodeBlock (RetNet-style decoder block).

Sharding: data-parallel over batch (B=8) across the 8 NeuronCores; each core
computes the full block for one batch element. No collectives.

Algorithm notes (per core, feature-major "transposed" dataflow):
  - All activations are kept feature-major: X^T [E=512(4 part-tiles), S=1024].
  - Retention decay D[h,n,m] = kappa_h^(n-m) (causal) is applied via global
    row/col scaling: qs^T = q^T * kappa^n, ks^T = k^T * kappa^-m, then a 0/1
    causal mask on diagonal blocks only (exact in fp32: kappa^-1023 <= 1.3e14).
  - scoresT[m,n] tiles come straight from PE with m on partitions; ret^T is
    accumulated per head with V (seq-major) as the stationary operand.
  - GroupNorm/RMSNorm stats are computed with ones-matmuls over partitions
    (PE) and broadcast back with gpsimd partition_broadcast.
  - Matmuls run as float32r (TF32-like) at free-dim 512 -> 1 cycle/row.
"""

import numpy as np

import concourse.bass as bass
import concourse.mybir as mybir
import concourse.tile as tile
from concourse.bass_utils import run_bass_kernel_spmd

F32 = mybir.dt.float32
BF16 = mybir.dt.bfloat16
AF = mybir.ActivationFunctionType

E, H, B, S = 512, 8, 8, 1024
DH = E // H          # 64
P = 128
NF = E // P          # 4 feature tiles
NS = S // P          # 8 seq tiles
NH2 = S // 512       # 2 n-halves

N_CORES = 8


def _kappas():
    k = 1.0 - np.exp(np.linspace(np.log(1.0 / 32.0), np.log(1.0 / 512.0), H))
    return k.astype(np.float64)


def r(ap):
    return ap


def _build_consts(inputs):
    """Host-side constant tensors shared by all cores."""
    import ml_dtypes
    bf16 = ml_dtypes.bfloat16
    kap = _kappas()
    n = np.arange(S, dtype=np.float64)
    kq = np.empty((E, S), np.float64)
    kk = np.empty((E, S), np.float64)
    for h in range(H):
        kq[h * DH:(h + 1) * DH, :] = (kap[h] ** n)[None, :]
        kk[h * DH:(h + 1) * DH, :] = (kap[h] ** (-n))[None, :]
    kqmap = np.ascontiguousarray(kq.astype(bf16))
    kkmap = np.ascontiguousarray(kk.astype(bf16))
    # causal mask for a [128, 4*512] psum group: section i covers m-block
    # offset 128*i vs n-block base: keep if j >= p + 128*i
    cmask = np.zeros((P, 4 * 512), np.float32)  # cast to bf16 below
    jj = np.arange(512)
    for i in range(4):
        cmask[:, i * 512:(i + 1) * 512] = (jj[None, :] >= (np.arange(P)[:, None] + 128 * i))
    cmask = cmask.astype(bf16)
    gn_ones = np.zeros((P, 2), bf16)
    gn_ones[:64, 0] = 1.0 / DH
    gn_ones[64:, 1] = 1.0 / DH
    gn_bcast = np.zeros((2, P), bf16)
    gn_bcast[0, :64] = 1.0
    gn_bcast[1, 64:] = 1.0
    rms_ones = np.zeros((P, 2), bf16)
    rms_ones[:, 0] = 1.0
    rms_bcast = np.zeros((2, P), bf16)
    rms_bcast[0, :] = 1.0
    ident = np.eye(P, dtype=np.float32)

    def pp(v):  # [512] -> [128, 4] per-partition layout
        return np.ascontiguousarray(np.asarray(v, np.float32).reshape(NF, P).T)

    consts = {
        "kqmap": kqmap, "kkmap": kkmap, "cmask": cmask,
        "gn_ones": gn_ones, "gn_bcast": gn_bcast, "rms_ones": rms_ones,
        "rms_bcast": rms_bcast, "ident": ident,
        "gs1_pp": pp(inputs["gs1"]), "gb1_pp": pp(inputs["gb1"]),
        "gs2_pp": pp(inputs["gs2"]), "gb2_pp": pp(inputs["gb2"]),
        "ln1_pp": pp(inputs["ln1_s"]), "ln2_pp": pp(inputs["ln2_s"]),
        "ln3_pp": pp(inputs["ln3_s"]),
        "rl1": np.vstack([np.asarray(inputs["ln1_s"], np.float32),
                          np.zeros(E, np.float32)]).astype(bf16),
        "rl2": np.vstack([np.asarray(inputs["ln2_s"], np.float32),
                          np.zeros(E, np.float32)]).astype(bf16),
        "rl3": np.vstack([np.asarray(inputs["ln3_s"], np.float32),
                          np.zeros(E, np.float32)]).astype(bf16),
    }
    for nm in ("wq", "wk", "wv"):
        for i in (1, 2):
            w = np.asarray(inputs[f"{nm}{i}"], np.float32)      # [H, E, DH]
            consts[f"{nm}c{i}"] = np.ascontiguousarray(
                w.transpose(1, 0, 2).reshape(E, E).astype(bf16))
    for nm in ("wg1", "wo1", "wg2", "wo2", "ffn_w_gate", "ffn_w_lin", "ffn_w_out"):
        consts[nm] = np.ascontiguousarray(np.asarray(inputs[nm], np.float32).astype(bf16))
    return consts


class _Prog:
    pass


def _build_program():
    nc = bass.Bass()
    pr = _Prog()
    pr.nc = nc
    d = {}
    d["x"] = nc.dram_tensor("x", [S, E], F32, kind="ExternalInput")
    d["obs"] = nc.dram_tensor("obs", [S, E], F32, kind="ExternalInput")
    for nm in ("wqc1", "wkc1", "wvc1", "wqc2", "wkc2", "wvc2",
               "wg1", "wo1", "wg2", "wo2",
               "ffn_w_gate", "ffn_w_lin", "ffn_w_out"):
        d[nm] = nc.dram_tensor(nm, [E, E], BF16, kind="ExternalInput")
    d["cmask"] = nc.dram_tensor("cmask", [P, 4 * 512], BF16, kind="ExternalInput")
    d["gn_ones"] = nc.dram_tensor("gn_ones", [P, 2], BF16, kind="ExternalInput")
    d["gn_bcast"] = nc.dram_tensor("gn_bcast", [2, P], BF16, kind="ExternalInput")
    d["rms_ones"] = nc.dram_tensor("rms_ones", [P, 2], BF16, kind="ExternalInput")
    d["rms_bcast"] = nc.dram_tensor("rms_bcast", [2, P], BF16, kind="ExternalInput")
    d["ident"] = nc.dram_tensor("ident", [P, P], F32, kind="ExternalInput")
    for nm in ("gs1_pp", "gb1_pp", "gs2_pp", "gb2_pp", "ln1_pp", "ln2_pp", "ln3_pp"):
        d[nm] = nc.dram_tensor(nm, [P, NF], F32, kind="ExternalInput")
    for nm in ("rl1", "rl2", "rl3"):
        d[nm] = nc.dram_tensor(nm, [2, E], BF16, kind="ExternalInput")
    d["kqmap"] = nc.dram_tensor("kqmap", [E, S], BF16, kind="ExternalInput")
    d["kkmap"] = nc.dram_tensor("kkmap", [E, S], BF16, kind="ExternalInput")
    out_h = nc.dram_tensor("out", [S, E], F32, kind="ExternalOutput")

    with tile.TileContext(nc) as tc:
        _emit(nc, tc, d, out_h)
    _strip_self_waits(nc)
    _legalize_wait_counts(nc)
    return pr


_ENGINE_PROC = {
    "PE": "PE", "DVE": "DVE", "Activation": "Activation",
    "Pool": "Pool", "SP": "SP",
}


def _strip_self_waits(nc):
    """Remove same-engine sem waits on engine compute instructions.

    Engines execute their FIFO in order (DVE/ACT drain between ops; PE only
    reorders LDWEIGHTS pull-ahead, and PE never writes SBUF), so a wait on
    the instruction's own engine semaphore is redundant — and walrus only
    allows 2 sync waits per instruction."""
    import concourse.mybir as mb
    for f in nc.m.functions:
        for blk in f.blocks:
            for inst in blk.instructions:
                si = getattr(inst, "sync_info", None)
                if si is None or not si.on_wait:
                    continue
                tname = type(inst).__name__
                if tname in ("InstDMACopy", "InstDrain", "InstEventSemaphore",
                             "InstTriggerDma"):
                    continue
                eng = getattr(inst, "engine", None)
                eng_name = getattr(eng, "name", str(eng))
                pref = {"PE": "PE_", "DVE": "DVE_", "Activation": "Activation_",
                        "Pool": "Pool_", "SP": "SP_"}.get(eng_name)
                if not pref:
                    continue
                kept = [w for w in si.on_wait if not str(w.ant_name).startswith(pref)]
                if len(kept) != len(si.on_wait):
                    si.on_wait = kept


def _bc(row_ap, n_part):
    """Partition-broadcast read AP: replicate a single-partition row across
    n_part partitions (partition-step-0 source, for DMA)."""
    return bass.AP(tensor=row_ap.tensor, offset=row_ap.offset,
                   ap=[[0, n_part]] + [list(p) for p in row_ap.ap[1:]])


_MAX_WAITS = 1
_WAIT_BUDGET = {"InstActivation": 1, "InstDrain": 0}


def _legalize_wait_counts(nc):
    """walrus allows at most 2 sync waits per lowered instruction. Move any
    excess waits onto injected same-engine sequencer NOPs placed immediately
    before the offending instruction (program order on the engine's stream
    gates the instruction behind the NOP's waits)."""
    import bass_rust
    import concourse.mybir as mb
    uid = [0]
    for f in nc.m.functions:
        for blk in f.blocks:
            insts = list(blk.instructions)
            out = []
            changed = False
            for inst in insts:
                si = getattr(inst, "sync_info", None)
                waits = list(si.on_wait) if si and si.on_wait else []
                plain = [w for w in waits if w.sync_type == "semaphore"]
                other = [w for w in waits if w.sync_type != "semaphore"]
                cap = _WAIT_BUDGET.get(type(inst).__name__, _MAX_WAITS)
                if len(plain) + len(other) > cap and len(plain) > 0:
                    budget = max(0, cap - len(other))
                    keep, excess = plain[:budget], plain[budget:]
                    while excess:
                        chunk, excess = excess[:1], excess[1:]
                        nop = bass_rust.InstNoOp(name=f"wnop-{uid[0]}", ins=[], outs=[])
                        uid[0] += 1
                        nop.engine = inst.engine
                        nop.sync_info = mb.SyncInfo(on_wait=chunk, on_update=[])
                        out.append(nop)
                    si.on_wait = other + keep
                    changed = True
                out.append(inst)
            if changed:
                blk.instructions = out


def _emit(nc, tc, d, out_h):
    from contextlib import ExitStack
    ctx = ExitStack()
    with ctx:
        # Pools. Wait-limit discipline: every instruction may carry at most 2
        # sync waits after walrus lowering, so each tile has a single writer
        # engine and PSUM pools are split by evacuating engine (pg_d -> DVE,
        # pg_a -> ACT).
        p_const = ctx.enter_context(tc.tile_pool(name="const", bufs=1))
        p_act = ctx.enter_context(tc.tile_pool(name="act", bufs=1))
        p_w = ctx.enter_context(tc.tile_pool(name="w", bufs=12))
        p_map = ctx.enter_context(tc.tile_pool(name="map", bufs=5))
        p_sc = ctx.enter_context(tc.tile_pool(name="sc", bufs=5))
        p_sq = ctx.enter_context(tc.tile_pool(name="sq", bufs=2))
        p_sm = ctx.enter_context(tc.tile_pool(name="sm", bufs=8))
        # note: ld pool shares output tiles
        p_ld = ctx.enter_context(tc.tile_pool(name="ld", bufs=4))
        pg_d = ctx.enter_context(tc.tile_pool(name="pgd", bufs=1, space="PSUM"))
        pg_a = ctx.enter_context(tc.tile_pool(name="pga", bufs=2, space="PSUM"))
        psc = ctx.enter_context(tc.tile_pool(name="psc", bufs=2, space="PSUM"))
        pret = ctx.enter_context(tc.tile_pool(name="pret", bufs=1, space="PSUM"))

        # ---- constants ----
        cmask = p_const.tile([P, 4 * 512], BF16)
        nc.sync.dma_start(out=cmask, in_=d["cmask"][:, :])
        gn_ones = p_const.tile([P, 2], BF16)
        nc.sync.dma_start(out=gn_ones, in_=d["gn_ones"][:, :])
        gn_bcast = p_const.tile([2, P], BF16)
        nc.sync.dma_start(out=gn_bcast, in_=d["gn_bcast"][:, :])
        rms_ones = p_const.tile([P, 2], BF16)
        nc.sync.dma_start(out=rms_ones, in_=d["rms_ones"][:, :])
        rms_bcast = p_const.tile([2, P], BF16)
        nc.sync.dma_start(out=rms_bcast, in_=d["rms_bcast"][:, :])
        ident = p_const.tile([P, P], F32)
        nc.sync.dma_start(out=ident, in_=d["ident"][:, :])
        ppv = {}
        for nm in ("gs1_pp", "gb1_pp", "gs2_pp", "gb2_pp", "ln1_pp", "ln2_pp", "ln3_pp"):
            t = p_const.tile([P, NF], F32, name=nm)
            nc.sync.dma_start(out=t, in_=d[nm][:, :])
            ppv[nm] = t
        rlv = {}
        for nm in ("rl1", "rl2", "rl3"):
            t = p_const.tile([2, E], BF16, name=nm)
            nc.sync.dma_start(out=t, in_=d[nm][:, :])
            rlv[nm] = t
        eps_gn = p_const.tile([P, 1], F32)
        nc.vector.memset(eps_gn, 1e-5)
        eps_rms = p_const.tile([P, 1], F32)
        nc.vector.memset(eps_rms, 1e-6)
        zero_d = p_const.tile([P, 1], F32)
        nc.vector.memset(zero_d, 0.0)
        # DVE observers for HWDGE const queues (keeps later DVE ops <=2 waits)
        wuv = p_const.tile([P, 1], F32)
        for cn in list(ppv.values()) + [cmask]:
            nc.vector.tensor_copy(wuv, cn[:, 0:1])

        # Warmup matmuls so PE observes each PE-read constant's DMA queue sem
        # early (keeps later matmuls at <=2 waits).
        wu = pg_d.tile([P, P], F32, tag="pgd", name="wu")
        nc.tensor.matmul(wu[0:2, 0:P], gn_ones, cmask[:, 0:P], start=True, stop=True)
        nc.tensor.matmul(wu[0:P, 0:P], gn_bcast, gn_bcast, start=False, stop=True,
                         skip_group_check=True)
        nc.tensor.matmul(wu[0:2, 0:P], rms_ones, cmask[:, 0:P], start=False, stop=True,
                         skip_group_check=True)
        nc.tensor.matmul(wu[0:P, 0:P], rms_bcast, rms_bcast, start=False, stop=True,
                         skip_group_check=True)

        def load_w(nm, tag="w"):
            tiles = []
            for k in range(NF):
                wt = p_w.tile([P, E], BF16, tag=tag, name=f"{nm}_{k}")
                nc.sync.dma_start(out=wt, in_=d[nm][k * P:(k + 1) * P, :])
                tiles.append(wt)
            return tiles

        # ---- phase 0: load + transpose x, obs -> xT, obsT (evac: DVE only) ----
        def transpose_in(src_h, out_tag):
            outT = []
            for k in range(NF):
                t = p_act.tile([P, S], BF16, tag=f"{out_tag}{k}", name=f"{out_tag}{k}")
                outT.append(t)
            for sidx in range(NS):
                s_sb = p_ld.tile([P, E], F32, tag="ld", name=f"ld_{sidx}")
                nc.sync.dma_start(out=s_sb, in_=src_h[sidx * P:(sidx + 1) * P, :])
                ps = pg_d.tile([P, E], F32, tag="pgd", name=f"tp_{sidx}")
                for k in range(NF):
                    nc.tensor.matmul(ps[:, k * P:(k + 1) * P], s_sb[:, k * P:(k + 1) * P],
                                     ident, is_transpose=True,
                                     start=(k == 0), stop=(k == NF - 1))
                for k in range(NF):
                    nc.scalar.copy(outT[k][:, sidx * P:(sidx + 1) * P],
                                   ps[:, k * P:(k + 1) * P])
            return outT

        xT = transpose_in(d["x"], "xT")
        obsT = transpose_in(d["obs"], "obsT")

        # ---- helper: [E,E] gemm, out feature-major: outT = W^T @ srcT ----
        def gemm_fm(w_tiles, srcT, evac, out_tag=None, out_tiles=None, out_dt=BF16,
                    psum_pool=None):
            pool = psum_pool or pg_d
            outs = out_tiles
            if outs is None:
                outs = [p_act.tile([P, S], out_dt, tag=f"{out_tag}{m}", name=f"{out_tag}{m}")
                        for m in range(NF)]
            for m in range(NF):
                for nh in range(NH2):
                    ps = pool.tile([P, 512], F32, tag=pool.name, name=f"g_{m}_{nh}")
                    for k in range(NF):
                        nc.tensor.matmul(
                            ps, r(w_tiles[k][:, m * P:(m + 1) * P]),
                            r(srcT[k][:, nh * 512:(nh + 1) * 512]),
                            start=(k == 0), stop=(k == NF - 1))
                    evac(outs[m][:, nh * 512:(nh + 1) * 512], ps, m, nh)
            return outs

        def msr(qsrcT, kvsrcT, wq_t, wk_t, wv_t, wg_t, wo_t, gs_pp, gb_pp, out_tiles,
                tap="", dump_fm=None):
            # q^T / k^T: plain DVE evac, then in-place decay-map multiply
            # (maps precomputed on host, streamed from HBM).
            def mk_evac_map(map_h):
                def evac(dst, ps, m, nh):
                    mt = p_map.tile([P, 512], BF16, tag="map", name=f"map_{m}")
                    nc.sync.dma_start(
                        out=mt, in_=map_h[m * P:(m + 1) * P, nh * 512:(nh + 1) * 512])
                    nc.vector.tensor_mul(dst, ps, mt)
                return evac

            qT = gemm_fm(wq_t, qsrcT, mk_evac_map(d["kqmap"]), out_tag="qT")
            if tap == "qT":
                dump_fm(qT)
                return
            kT = gemm_fm(wk_t, kvsrcT, mk_evac_map(d["kkmap"]), out_tag="kT")
            if tap == "kT":
                dump_fm(kT)
                return

            # V seq-major: V[st] [128, 512(all heads)]
            V = []
            for st in range(NS):
                ps = pg_d.tile([P, 512], F32, tag="pgd", name=f"v_{st}")
                for k in range(NF):
                    nc.tensor.matmul(ps, r(kvsrcT[k][:, st * P:(st + 1) * P]), r(wv_t[k]),
                                     start=(k == 0), stop=(k == NF - 1))
                vt = p_act.tile([P, 512], BF16, tag=f"V{st}", name=f"V{st}")
                nc.scalar.copy(vt, ps)
                V.append(vt)
            if tap == "V":
                dump_fm(V, n_tiles=NS, width=E)
                return

            # scores + ret; two heads (one pair tile) share a ret psum bank:
            # even head -> rows 0:64, odd head -> rows 64:128 (col group).
            retT = [p_act.tile([P, S], BF16, tag=f"retT{pt}", name=f"retT{pt}") for pt in range(NF)]
            for pt in range(NF):
                for nt in range(NH2):
                    prt = pret.tile([P, 512], F32, tag="pret", name=f"pret_{pt}_{nt}")
                    groups = ([[0, 1], [2, 3]] if nt == 0
                              else [[0, 1], [2, 3], [4, 5], [6, 7]])
                    n_head_mm = sum(len(g) for g in groups)
                    for hh in range(2):      # head within pair
                        mm_i = 0
                        h = pt * 2 + hh
                        sl = hh * 64
                        for gi, grp in enumerate(groups):
                            ps4 = psc.tile([P, 2 * 512], F32, tag="psc", name=f"sc_{h}_{nt}_{gi}")
                            for j, mt in enumerate(grp):
                                # each j targets its own PSUM bank -> own group
                                nc.tensor.matmul(
                                    ps4[:, j * 512:(j + 1) * 512],
                                    r(kT[pt][sl:sl + 64, mt * P:(mt + 1) * P]),
                                    r(qT[pt][sl:sl + 64, nt * 512:(nt + 1) * 512]),
                                    start=True, stop=True)
                            sc_sb = p_sc.tile([P, 2 * 512], BF16, tag="scsb", name=f"scsb_{h}_{nt}_{gi}")
                            masked = (grp[-1] * P + P - 1) >= nt * 512
                            if masked:
                                for j, mt in enumerate(grp):
                                    off = mt * P - nt * 512   # 0/128/256/384
                                    s0 = j * 512
                                    if off > 0:
                                        nc.gpsimd.memset(sc_sb[:, s0:s0 + off], 0.0)
                                    # diagonal block: mask-multiply (cmask diag
                                    # of section i=off//128 is at abs col
                                    # i*512 + off)
                                    ci = (off // 128) * 512 + off
                                    nc.vector.tensor_mul(
                                        sc_sb[:, s0 + off:s0 + off + P],
                                        ps4[:, s0 + off:s0 + off + P],
                                        cmask[:, ci:ci + P])
                                    if off + P < 512:
                                        nc.scalar.copy(
                                            sc_sb[:, s0 + off + P:s0 + 512],
                                            ps4[:, s0 + off + P:s0 + 512])
                            else:
                                nc.scalar.copy(sc_sb, ps4)
                            for j, mt in enumerate(grp):
                                nc.tensor.matmul(
                                    prt[sl:sl + 64, :],
                                    r(V[mt][:, h * DH:(h + 1) * DH]),
                                    r(sc_sb[:, j * 512:(j + 1) * 512]),
                                    start=(mm_i == 0), stop=(mm_i == n_head_mm - 1),
                                    tile_position=(0, sl), skip_group_check=True)
                                mm_i += 1
                    nc.vector.tensor_copy(retT[pt][:, nt * 512:(nt + 1) * 512], prt)

            if tap == "ret":
                dump_fm(retT)
                return
            # GroupNorm (feature-major, stats over 64 partitions per head).
            # Small tiles: gnA (DVE: mu rows0-1, var rows2-3), gnB (DVE mu^2
            # rows0-1 / ACT sd rows2-3), gnC (DVE rstd rows0-1, feeds PE).
            for pt in range(NF):
                for nt in range(NH2):
                    rsl = retT[pt][:, nt * 512:(nt + 1) * 512]
                    sqt = p_sq.tile([P, 512], BF16, tag="gnsq", name=f"gnsq_{pt}_{nt}", bufs=2)
                    nc.gpsimd.tensor_mul(sqt, rsl, rsl)
                    pstat = pg_d.tile([P, 512], F32, tag="pgd", name=f"gst_{pt}_{nt}")
                    nc.tensor.matmul(pstat[0:2, :], r(gn_ones), r(rsl), start=True, stop=True)
                    pstat2 = pg_d.tile([P, 512], F32, tag="pgd", name=f"gst2_{pt}_{nt}")
                    nc.tensor.matmul(pstat2[0:2, :], r(gn_ones), r(sqt), start=True, stop=True)
                    mu = p_sm.tile([2, 512], BF16, tag="sm", name=f"mu_{pt}_{nt}")
                    mu2 = p_sm.tile([2, 512], F32, tag="sm", name=f"mu2_{pt}_{nt}")
                    var = p_sm.tile([2, 512], F32, tag="sm", name=f"var_{pt}_{nt}")
                    sd = p_sm.tile([2, 512], F32, tag="sm", name=f"sd_{pt}_{nt}")
                    rstd = p_sm.tile([2, 512], BF16, tag="sm", name=f"rstd_{pt}_{nt}")
                    nc.vector.tensor_copy(mu, pstat[0:2, :])
                    nc.vector.tensor_mul(mu2, mu, mu)
                    nc.vector.tensor_sub(var, pstat2[0:2, :], mu2)
                    nc.scalar.activation(sd, var, AF.Sqrt, bias=eps_gn[0:2, :])
                    with nc.allow_low_precision(reason="rstd feeds bf16 broadcast matmul"):
                        nc.vector.reciprocal(rstd, sd)
                    muBp = pg_d.tile([P, 512], F32, tag="pgd", name=f"muBp_{pt}_{nt}")
                    nc.tensor.matmul(muBp, r(gn_bcast), r(mu), start=True, stop=True)
                    rsBp = pg_d.tile([P, 512], F32, tag="pgd", name=f"rsBp_{pt}_{nt}")
                    nc.tensor.matmul(rsBp, r(gn_bcast), r(rstd), start=True, stop=True)
                    nc.vector.tensor_sub(rsl, rsl, muBp)
                    nc.vector.tensor_mul(rsl, rsl, rsBp)
                    nc.scalar.activation(rsl, rsl, AF.Identity,
                                         bias=gb_pp[:, pt:pt + 1], scale=gs_pp[:, pt:pt + 1])

            # gate: g^T = silu(Wg^T @ qsrcT); silu evac on ACT from pg_a
            def evac_g(dst, ps, m, nh):
                nc.scalar.activation(dst, ps, AF.Silu)

            gT = gemm_fm(wg_t, qsrcT, evac_g, out_tag="qT", psum_pool=pg_a)
            # gated = swish(g) * retGN, written into retT (PE reads retT for wo)
            for m in range(NF):
                nc.gpsimd.tensor_mul(retT[m], gT[m], retT[m])

            def evac_o(dst, ps, m, nh):
                nc.scalar.copy(dst, ps)

            gemm_fm(wo_t, retT, evac_o, out_tiles=out_tiles, psum_pool=pg_a)

        # feature-major RMSNorm: out = (a + b) * rsqrt(mean_f((a+b)^2) + eps) * ln
        def rms_fm(aT, bT, ln_pp, out_tag=None, out_tiles=None, out_dt=BF16):
            res = bT
            for k in range(NF):
                nc.vector.tensor_add(res[k], aT[k], bT[k])
            outs = out_tiles
            if outs is None:
                outs = [p_act.tile([P, S], out_dt, tag=f"{out_tag}{k}", name=f"{out_tag}{k}")
                        for k in range(NF)]
            for nh in range(NH2):
                pstat = pg_d.tile([P, 512], F32, tag="pgd", name=f"rst_{nh}")
                for k in range(NF):
                    sqt = p_sq.tile([P, 512], BF16, tag="sq", name=f"rsq_{nh}_{k}")
                    rs = res[k][:, nh * 512:(nh + 1) * 512]
                    nc.vector.tensor_mul(sqt, rs, rs)
                    nc.tensor.matmul(pstat[0:2, :], r(rms_ones), r(sqt),
                                     start=(k == 0), stop=(k == NF - 1))
                r0 = p_sm.tile([2, 512], F32, tag="sm", name=f"r0_{nh}")
                nc.vector.tensor_copy(r0[0:1, :], pstat[0:1, :])
                rA = p_sm.tile([2, 512], F32, tag="sm", name=f"rA_{nh}")
                nc.scalar.activation(rA[0:1, :], r0[0:1, :], AF.Sqrt,
                                     bias=eps_rms[0:1, :], scale=1.0 / E)
                rB = p_sm.tile([2, 512], BF16, tag="sm", name=f"rB_{nh}")
                nc.vector.memset(rB, 0.0)
                with nc.allow_low_precision(reason="rstd feeds bf16 broadcast matmul"):
                    nc.vector.reciprocal(rB[0:1, :], rA[0:1, :])
                rsB = pg_d.tile([P, 512], F32, tag="pgd", name=f"rmsB_{nh}")
                nc.tensor.matmul(rsB, r(rms_bcast), r(rB[0:2, :]), start=True, stop=True)
                for k in range(NF):
                    osl = outs[k][:, nh * 512:(nh + 1) * 512]
                    nc.vector.tensor_mul(osl, res[k][:, nh * 512:(nh + 1) * 512], rsB)
                    nc.gpsimd.tensor_scalar_mul(osl, osl, ln_pp[:, k:k + 1])
            return outs

        # ======== the block ========
        import os
        tap = os.environ.get("KTAP", "")

        def dump_fm(tiles, n_tiles=NF, width=S):
            # write feature-major tiles [128, width] into out rows sequentially
            for k in range(n_tiles):
                t32 = p_ld.tile([P, S], F32, tag="dump", name=f"dmp_{k}", bufs=2)
                nc.vector.tensor_copy(t32[:, :width], tiles[k][:, :width])
                rows = width // E
                for rr in range(rows):
                    nc.sync.dma_start(
                        out=out_h[(k * rows + rr) * P:(k * rows + rr + 1) * P, :],
                        in_=t32[:, rr * E:(rr + 1) * E])
        wq1 = load_w("wqc1"); wk1 = load_w("wkc1"); wv1 = load_w("wvc1")
        wg1 = load_w("wg1"); wo1 = load_w("wo1")
        msr1T = [p_act.tile([P, S], F32, tag=f"msrT{m}", name=f"msr1T{m}") for m in range(NF)]
        if tap == "xT":
            dump_fm(xT)
            return
        msr(xT, xT, wq1, wk1, wv1, wg1, wo1, ppv["gs1_pp"], ppv["gb1_pp"], msr1T,
            tap=tap, dump_fm=dump_fm)
        if tap:
            if tap == "msr1":
                dump_fm(msr1T)
            if tap in ("msr1", "qT", "kT", "V", "ret"):
                return
        x1T = rms_fm(xT, msr1T, ppv["ln1_pp"], out_tag="x1T")
        if tap == "x1":
            dump_fm(x1T)
            return

        wq2 = load_w("wqc2"); wk2 = load_w("wkc2"); wv2 = load_w("wvc2")
        wg2 = load_w("wg2"); wo2 = load_w("wo2")
        msr2T = [p_act.tile([P, S], F32, tag=f"msrT{m}", name=f"msr2T{m}") for m in range(NF)]
        msr(obsT, x1T, wq2, wk2, wv2, wg2, wo2, ppv["gs2_pp"], ppv["gb2_pp"], msr2T)
        x2T = rms_fm(obsT, msr2T, ppv["ln2_pp"], out_tag="xT")  # reuse xT slots

        def load_w_tags(nm, tags):
            tiles = []
            for k in range(NF):
                wt = p_act.tile([P, E], BF16, tag=tags[k], name=f"{nm}_{k}")
                nc.sync.dma_start(out=wt, in_=d[nm][k * P:(k + 1) * P, :])
                tiles.append(wt)
            return tiles

        wfg = load_w_tags("ffn_w_gate", [f"V{i}" for i in range(4)])
        wfl = load_w("ffn_w_lin")
        wfo = load_w("ffn_w_out")

        def evac_silu(dst, ps, m, nh):
            nc.scalar.activation(dst, ps, AF.Silu)

        def evac_cp_d(dst, ps, m, nh):
            nc.scalar.copy(dst, ps)

        fgT = gemm_fm(wfg, x2T, evac_silu, out_tag="qT", psum_pool=pg_a)
        flT = gemm_fm(wfl, x2T, evac_cp_d, out_tag="kT", psum_pool=pg_a)
        for m in range(NF):
            nc.gpsimd.tensor_mul(flT[m], fgT[m], flT[m])
        ffnT = gemm_fm(wfo, flT, evac_cp_d, out_tag="x1T", out_dt=F32, psum_pool=pg_a)
        x3T = rms_fm(x2T, ffnT, ppv["ln3_pp"], out_tag="msrT", out_dt=F32)

        # ---- output transpose: x3T [E,S] -> out [S,E] ----
        for sidx in range(NS):
            ps = pg_d.tile([P, E], F32, tag="pgd", name=f"ot_{sidx}")
            for k in range(NF):
                nc.tensor.matmul(ps[:, k * P:(k + 1) * P],
                                 x3T[k][:, sidx * P:(sidx + 1) * P], ident,
                                 is_transpose=True,
                                 start=(k == 0), stop=(k == NF - 1))
            ot = p_ld.tile([P, E], F32, tag="ot", name=f"ot_{sidx}", bufs=1)
            nc.vector.tensor_copy(ot, ps)
            nc.sync.dma_start(out=out_h[sidx * P:(sidx + 1) * P, :], in_=ot)


_prog_cache = None


def _get_program():
    global _prog_cache
    if _prog_cache is None:
        _prog_cache = _build_program()
    return _prog_cache


def kernel(**inputs):
    inputs = {k: np.asarray(v) for k, v in inputs.items()}
    consts = _build_consts(inputs)
    pr = _get_program()
    shared = dict(consts)
    x = np.ascontiguousarray(inputs["x"], dtype=np.float32)
    obs = np.ascontiguousarray(inputs["obs_rep"], dtype=np.float32)
    in_maps = []
    for b in range(N_CORES):
        m = dict(shared)
        m["x"] = np.ascontiguousarray(x[b])
        m["obs"] = np.ascontiguousarray(obs[b])
        in_maps.append(m)
    res = run_bass_kernel_spmd(pr.nc, in_maps, core_ids=list(range(N_CORES)))
    return np.stack([res.results[b]["out"] for b in range(N_CORES)], axis=0)


# revision 56
# speedup vs baseline: 1.1461x; 1.1461x over previous
"""Trainium2 Bass kernel for nn_DecodeBlock (RetNet-style decoder block).

Sharding: data-parallel over batch (B=8) across the 8 NeuronCores; each core
computes the full block for one batch element. No collectives.

Algorithm notes (per core, feature-major "transposed" dataflow):
  - All activations are kept feature-major: X^T [E=512(4 part-tiles), S=1024].
  - Retention decay D[h,n,m] = kappa_h^(n-m) (causal) is applied via global
    row/col scaling: qs^T = q^T * kappa^n, ks^T = k^T * kappa^-m, then a 0/1
    causal mask on diagonal blocks only (exact in fp32: kappa^-1023 <= 1.3e14).
  - scoresT[m,n] tiles come straight from PE with m on partitions; ret^T is
    accumulated per head with V (seq-major) as the stationary operand.
  - GroupNorm/RMSNorm stats are computed with ones-matmuls over partitions
    (PE) and broadcast back with gpsimd partition_broadcast.
  - Matmuls run as float32r (TF32-like) at free-dim 512 -> 1 cycle/row.
"""

import numpy as np

import concourse.bass as bass
import concourse.mybir as mybir
import concourse.tile as tile
from concourse.bass_utils import run_bass_kernel_spmd

F32 = mybir.dt.float32
BF16 = mybir.dt.bfloat16
AF = mybir.ActivationFunctionType

E, H, B, S = 512, 8, 8, 1024
DH = E // H          # 64
P = 128
NF = E // P          # 4 feature tiles
NS = S // P          # 8 seq tiles
NH2 = S // 512       # 2 n-halves

N_CORES = 8


def _kappas():
    k = 1.0 - np.exp(np.linspace(np.log(1.0 / 32.0), np.log(1.0 / 512.0), H))
    return k.astype(np.float64)


def r(ap):
    return ap


def _build_consts(inputs):
    """Host-side constant tensors shared by all cores."""
    import ml_dtypes
    bf16 = ml_dtypes.bfloat16
    kap = _kappas()
    n = np.arange(S, dtype=np.float64)
    kq = np.empty((E, S), np.float64)
    kk = np.empty((E, S), np.float64)
    for h in range(H):
        kq[h * DH:(h + 1) * DH, :] = (kap[h] ** n)[None, :]
        kk[h * DH:(h + 1) * DH, :] = (kap[h] ** (-n))[None, :]
    kqmap = np.ascontiguousarray(kq.astype(bf16))
    kkmap = np.ascontiguousarray(kk.astype(bf16))
    # causal mask for a [128, 4*512] psum group: section i covers m-block
    # offset 128*i vs n-block base: keep if j >= p + 128*i
    cmask = np.zeros((P, 4 * 512), np.float32)  # cast to bf16 below
    jj = np.arange(512)
    for i in range(4):
        cmask[:, i * 512:(i + 1) * 512] = (jj[None, :] >= (np.arange(P)[:, None] + 128 * i))
    cmask = cmask.astype(bf16)
    gn_ones = np.zeros((P, 2), bf16)
    gn_ones[:64, 0] = 1.0 / DH
    gn_ones[64:, 1] = 1.0 / DH
    gn_bcast = np.zeros((2, P), bf16)
    gn_bcast[0, :64] = 1.0
    gn_bcast[1, 64:] = 1.0
    rms_ones = np.zeros((P, 2), bf16)
    rms_ones[:, 0] = 1.0
    rms_bcast = np.zeros((2, P), bf16)
    rms_bcast[0, :] = 1.0
    ident = np.eye(P, dtype=np.float32)

    def pp(v):  # [512] -> [128, 4] per-partition layout
        return np.ascontiguousarray(np.asarray(v, np.float32).reshape(NF, P).T)

    consts = {
        "kqmap": kqmap, "kkmap": kkmap, "cmask": cmask,
        "gn_ones": gn_ones, "gn_bcast": gn_bcast, "rms_ones": rms_ones,
        "rms_bcast": rms_bcast, "ident": ident,
        "gs1_pp": pp(inputs["gs1"]), "gb1_pp": pp(inputs["gb1"]),
        "gs2_pp": pp(inputs["gs2"]), "gb2_pp": pp(inputs["gb2"]),
        "ln1_pp": pp(inputs["ln1_s"]), "ln2_pp": pp(inputs["ln2_s"]),
        "ln3_pp": pp(inputs["ln3_s"]),
        "rl1": np.vstack([np.asarray(inputs["ln1_s"], np.float32),
                          np.zeros(E, np.float32)]).astype(bf16),
        "rl2": np.vstack([np.asarray(inputs["ln2_s"], np.float32),
                          np.zeros(E, np.float32)]).astype(bf16),
        "rl3": np.vstack([np.asarray(inputs["ln3_s"], np.float32),
                          np.zeros(E, np.float32)]).astype(bf16),
    }
    for nm in ("wq", "wk", "wv"):
        for i in (1, 2):
            w = np.asarray(inputs[f"{nm}{i}"], np.float32)      # [H, E, DH]
            consts[f"{nm}c{i}"] = np.ascontiguousarray(
                w.transpose(1, 0, 2).reshape(E, E).astype(bf16))
    for nm in ("wg1", "wo1", "wg2", "wo2", "ffn_w_gate", "ffn_w_lin", "ffn_w_out"):
        consts[nm] = np.ascontiguousarray(np.asarray(inputs[nm], np.float32).astype(bf16))
    return consts


class _Prog:
    pass


def _build_program():
    nc = bass.Bass()
    pr = _Prog()
    pr.nc = nc
    d = {}
    d["x"] = nc.dram_tensor("x", [S, E], F32, kind="ExternalInput")
    d["obs"] = nc.dram_tensor("obs", [S, E], F32, kind="ExternalInput")
    for nm in ("wqc1", "wkc1", "wvc1", "wqc2", "wkc2", "wvc2",
               "wg1", "wo1", "wg2", "wo2",
               "ffn_w_gate", "ffn_w_lin", "ffn_w_out"):
        d[nm] = nc.dram_tensor(nm, [E, E], BF16, kind="ExternalInput")
    d["cmask"] = nc.dram_tensor("cmask", [P, 4 * 512], BF16, kind="ExternalInput")
    d["gn_ones"] = nc.dram_tensor("gn_ones", [P, 2], BF16, kind="ExternalInput")
    d["gn_bcast"] = nc.dram_tensor("gn_bcast", [2, P], BF16, kind="ExternalInput")
    d["rms_ones"] = nc.dram_tensor("rms_ones", [P, 2], BF16, kind="ExternalInput")
    d["rms_bcast"] = nc.dram_tensor("rms_bcast", [2, P], BF16, kind="ExternalInput")
    d["ident"] = nc.dram_tensor("ident", [P, P], F32, kind="ExternalInput")
    for nm in ("gs1_pp", "gb1_pp", "gs2_pp", "gb2_pp", "ln1_pp", "ln2_pp", "ln3_pp"):
        d[nm] = nc.dram_tensor(nm, [P, NF], F32, kind="ExternalInput")
    for nm in ("rl1", "rl2", "rl3"):
        d[nm] = nc.dram_tensor(nm, [2, E], BF16, kind="ExternalInput")
    d["kqmap"] = nc.dram_tensor("kqmap", [E, S], BF16, kind="ExternalInput")
    d["kkmap"] = nc.dram_tensor("kkmap", [E, S], BF16, kind="ExternalInput")
    out_h = nc.dram_tensor("out", [S, E], F32, kind="ExternalOutput")

    with tile.TileContext(nc) as tc:
        _emit(nc, tc, d, out_h)
    _strip_self_waits(nc)
    _legalize_wait_counts(nc)
    return pr


_ENGINE_PROC = {
    "PE": "PE", "DVE": "DVE", "Activation": "Activation",
    "Pool": "Pool", "SP": "SP",
}


def _strip_self_waits(nc):
    """Remove same-engine sem waits on engine compute instructions.

    Engines execute their FIFO in order (DVE/ACT drain between ops; PE only
    reorders LDWEIGHTS pull-ahead, and PE never writes SBUF), so a wait on
    the instruction's own engine semaphore is redundant — and walrus only
    allows 2 sync waits per instruction."""
    import concourse.mybir as mb
    for f in nc.m.functions:
        for blk in f.blocks:
            for inst in blk.instructions:
                si = getattr(inst, "sync_info", None)
                if si is None or not si.on_wait:
                    continue
                tname = type(inst).__name__
                if tname in ("InstDMACopy", "InstDrain", "InstEventSemaphore",
                             "InstTriggerDma"):
                    continue
                eng = getattr(inst, "engine", None)
                eng_name = getattr(eng, "name", str(eng))
                pref = {"PE": "PE_", "DVE": "DVE_", "Activation": "Activation_",
                        "Pool": "Pool_", "SP": "SP_"}.get(eng_name)
                if not pref:
                    continue
                kept = [w for w in si.on_wait if not str(w.ant_name).startswith(pref)]
                if len(kept) != len(si.on_wait):
                    si.on_wait = kept


def _bc(row_ap, n_part):
    """Partition-broadcast read AP: replicate a single-partition row across
    n_part partitions (partition-step-0 source, for DMA)."""
    return bass.AP(tensor=row_ap.tensor, offset=row_ap.offset,
                   ap=[[0, n_part]] + [list(p) for p in row_ap.ap[1:]])


_MAX_WAITS = 1
_WAIT_BUDGET = {"InstActivation": 1, "InstDrain": 0}


def _legalize_wait_counts(nc):
    """walrus allows at most 2 sync waits per lowered instruction. Move any
    excess waits onto injected same-engine sequencer NOPs placed immediately
    before the offending instruction (program order on the engine's stream
    gates the instruction behind the NOP's waits)."""
    import bass_rust
    import concourse.mybir as mb
    uid = [0]
    for f in nc.m.functions:
        for blk in f.blocks:
            insts = list(blk.instructions)
            out = []
            changed = False
            for inst in insts:
                si = getattr(inst, "sync_info", None)
                waits = list(si.on_wait) if si and si.on_wait else []
                plain = [w for w in waits if w.sync_type == "semaphore"]
                other = [w for w in waits if w.sync_type != "semaphore"]
                cap = _WAIT_BUDGET.get(type(inst).__name__, _MAX_WAITS)
                if len(plain) + len(other) > cap and len(plain) > 0:
                    budget = max(0, cap - len(other))
                    keep, excess = plain[:budget], plain[budget:]
                    while excess:
                        chunk, excess = excess[:1], excess[1:]
                        nop = bass_rust.InstNoOp(name=f"wnop-{uid[0]}", ins=[], outs=[])
                        uid[0] += 1
                        nop.engine = inst.engine
                        nop.sync_info = mb.SyncInfo(on_wait=chunk, on_update=[])
                        out.append(nop)
                    si.on_wait = other + keep
                    changed = True
                out.append(inst)
            if changed:
                blk.instructions = out


def _emit(nc, tc, d, out_h):
    from contextlib import ExitStack
    ctx = ExitStack()
    with ctx:
        # Pools. Wait-limit discipline: every instruction may carry at most 2
        # sync waits after walrus lowering, so each tile has a single writer
        # engine and PSUM pools are split by evacuating engine (pg_d -> DVE,
        # pg_a -> ACT).
        p_const = ctx.enter_context(tc.tile_pool(name="const", bufs=1))
        p_act = ctx.enter_context(tc.tile_pool(name="act", bufs=1))
        p_w = ctx.enter_context(tc.tile_pool(name="w", bufs=12))
        p_map = ctx.enter_context(tc.tile_pool(name="map", bufs=5))
        p_sc = ctx.enter_context(tc.tile_pool(name="sc", bufs=5))
        p_sq = ctx.enter_context(tc.tile_pool(name="sq", bufs=2))
        p_sm = ctx.enter_context(tc.tile_pool(name="sm", bufs=8))
        # note: ld pool shares output tiles
        p_ld = ctx.enter_context(tc.tile_pool(name="ld", bufs=4))
        pg_d = ctx.enter_context(tc.tile_pool(name="pgd", bufs=2, space="PSUM"))
        pg_a = ctx.enter_context(tc.tile_pool(name="pga", bufs=2, space="PSUM"))
        psc = ctx.enter_context(tc.tile_pool(name="psc", bufs=3, space="PSUM"))
        pret = ctx.enter_context(tc.tile_pool(name="pret", bufs=1, space="PSUM"))

        # ---- constants ----
        cmask = p_const.tile([P, 4 * 512], BF16)
        nc.sync.dma_start(out=cmask, in_=d["cmask"][:, :])
        gn_ones = p_const.tile([P, 2], BF16)
        nc.sync.dma_start(out=gn_ones, in_=d["gn_ones"][:, :])
        gn_bcast = p_const.tile([2, P], BF16)
        nc.sync.dma_start(out=gn_bcast, in_=d["gn_bcast"][:, :])
        rms_ones = p_const.tile([P, 2], BF16)
        nc.sync.dma_start(out=rms_ones, in_=d["rms_ones"][:, :])
        rms_bcast = p_const.tile([2, P], BF16)
        nc.sync.dma_start(out=rms_bcast, in_=d["rms_bcast"][:, :])
        ident = p_const.tile([P, P], F32)
        nc.sync.dma_start(out=ident, in_=d["ident"][:, :])
        ppv = {}
        for nm in ("gs1_pp", "gb1_pp", "gs2_pp", "gb2_pp", "ln1_pp", "ln2_pp", "ln3_pp"):
            t = p_const.tile([P, NF], F32, name=nm)
            nc.sync.dma_start(out=t, in_=d[nm][:, :])
            ppv[nm] = t
        rlv = {}
        for nm in ("rl1", "rl2", "rl3"):
            t = p_const.tile([2, E], BF16, name=nm)
            nc.sync.dma_start(out=t, in_=d[nm][:, :])
            rlv[nm] = t
        eps_gn = p_const.tile([P, 1], F32)
        nc.vector.memset(eps_gn, 1e-5)
        eps_rms = p_const.tile([P, 1], F32)
        nc.vector.memset(eps_rms, 1e-6)
        zero_d = p_const.tile([P, 1], F32)
        nc.vector.memset(zero_d, 0.0)
        # DVE observers for HWDGE const queues (keeps later DVE ops <=2 waits)
        wuv = p_const.tile([P, 1], F32)
        for cn in list(ppv.values()) + [cmask]:
            nc.vector.tensor_copy(wuv, cn[:, 0:1])

        # Warmup matmuls so PE observes each PE-read constant's DMA queue sem
        # early (keeps later matmuls at <=2 waits).
        wu = pg_d.tile([P, P], F32, tag="pgd", name="wu")
        nc.tensor.matmul(wu[0:2, 0:P], gn_ones, cmask[:, 0:P], start=True, stop=True)
        nc.tensor.matmul(wu[0:P, 0:P], gn_bcast, gn_bcast, start=False, stop=True,
                         skip_group_check=True)
        nc.tensor.matmul(wu[0:2, 0:P], rms_ones, cmask[:, 0:P], start=False, stop=True,
                         skip_group_check=True)
        nc.tensor.matmul(wu[0:P, 0:P], rms_bcast, rms_bcast, start=False, stop=True,
                         skip_group_check=True)

        def load_w(nm, tag="w"):
            tiles = []
            for k in range(NF):
                wt = p_w.tile([P, E], BF16, tag=tag, name=f"{nm}_{k}")
                nc.sync.dma_start(out=wt, in_=d[nm][k * P:(k + 1) * P, :])
                tiles.append(wt)
            return tiles

        # ---- phase 0: load + transpose x, obs -> xT, obsT (evac: DVE only) ----
        def transpose_in(src_h, out_tag):
            outT = []
            for k in range(NF):
                t = p_act.tile([P, S], BF16, tag=f"{out_tag}{k}", name=f"{out_tag}{k}")
                outT.append(t)
            for sidx in range(NS):
                s_sb = p_ld.tile([P, E], F32, tag="ld", name=f"ld_{sidx}")
                nc.sync.dma_start(out=s_sb, in_=src_h[sidx * P:(sidx + 1) * P, :])
                ps = pg_d.tile([P, E], F32, tag="pgd", name=f"tp_{sidx}")
                for k in range(NF):
                    nc.tensor.matmul(ps[:, k * P:(k + 1) * P], s_sb[:, k * P:(k + 1) * P],
                                     ident, is_transpose=True,
                                     start=(k == 0), stop=(k == NF - 1))
                for k in range(NF):
                    nc.scalar.copy(outT[k][:, sidx * P:(sidx + 1) * P],
                                   ps[:, k * P:(k + 1) * P])
            return outT

        xT = transpose_in(d["x"], "xT")
        obsT = transpose_in(d["obs"], "obsT")

        # ---- helper: [E,E] gemm, out feature-major: outT = W^T @ srcT ----
        def gemm_fm(w_tiles, srcT, evac, out_tag=None, out_tiles=None, out_dt=BF16,
                    psum_pool=None):
            pool = psum_pool or pg_d
            outs = out_tiles
            if outs is None:
                outs = [p_act.tile([P, S], out_dt, tag=f"{out_tag}{m}", name=f"{out_tag}{m}")
                        for m in range(NF)]
            for m in range(NF):
                for nh in range(NH2):
                    ps = pool.tile([P, 512], F32, tag=pool.name, name=f"g_{m}_{nh}")
                    for k in range(NF):
                        nc.tensor.matmul(
                            ps, r(w_tiles[k][:, m * P:(m + 1) * P]),
                            r(srcT[k][:, nh * 512:(nh + 1) * 512]),
                            start=(k == 0), stop=(k == NF - 1))
                    evac(outs[m][:, nh * 512:(nh + 1) * 512], ps, m, nh)
            return outs

        def msr(qsrcT, kvsrcT, wq_t, wk_t, wv_t, wg_t, wo_t, gs_pp, gb_pp, out_tiles,
                tap="", dump_fm=None):
            # q^T / k^T: plain DVE evac, then in-place decay-map multiply
            # (maps precomputed on host, streamed from HBM).
            def mk_evac_map(map_h):
                def evac(dst, ps, m, nh):
                    mt = p_map.tile([P, 512], BF16, tag="map", name=f"map_{m}")
                    nc.sync.dma_start(
                        out=mt, in_=map_h[m * P:(m + 1) * P, nh * 512:(nh + 1) * 512])
                    nc.vector.tensor_mul(dst, ps, mt)
                return evac

            qT = gemm_fm(wq_t, qsrcT, mk_evac_map(d["kqmap"]), out_tag="qT")
            if tap == "qT":
                dump_fm(qT)
                return
            kT = gemm_fm(wk_t, kvsrcT, mk_evac_map(d["kkmap"]), out_tag="kT")
            if tap == "kT":
                dump_fm(kT)
                return

            # V seq-major: V[st] [128, 512(all heads)]
            V = []
            for st in range(NS):
                ps = pg_d.tile([P, 512], F32, tag="pgd", name=f"v_{st}")
                for k in range(NF):
                    nc.tensor.matmul(ps, r(kvsrcT[k][:, st * P:(st + 1) * P]), r(wv_t[k]),
                                     start=(k == 0), stop=(k == NF - 1))
                vt = p_act.tile([P, 512], BF16, tag=f"V{st}", name=f"V{st}")
                nc.scalar.copy(vt, ps)
                V.append(vt)
            if tap == "V":
                dump_fm(V, n_tiles=NS, width=E)
                return

            # scores + ret; two heads (one pair tile) share a ret psum bank:
            # even head -> rows 0:64, odd head -> rows 64:128 (col group).
            retT = [p_act.tile([P, S], BF16, tag=f"retT{pt}", name=f"retT{pt}") for pt in range(NF)]
            for pt in range(NF):
                for nt in range(NH2):
                    prt = pret.tile([P, 512], F32, tag="pret", name=f"pret_{pt}_{nt}")
                    groups = ([[0], [1], [2], [3]] if nt == 0
                              else [[0], [1], [2], [3], [4], [5], [6], [7]])
                    n_head_mm = sum(len(g) for g in groups)
                    for hh in range(2):      # head within pair
                        mm_i = 0
                        h = pt * 2 + hh
                        sl = hh * 64
                        for gi, grp in enumerate(groups):
                            ps4 = psc.tile([P, 512], F32, tag="psc", name=f"sc_{h}_{nt}_{gi}")
                            for j, mt in enumerate(grp):
                                # each j targets its own PSUM bank -> own group
                                nc.tensor.matmul(
                                    ps4[:, j * 512:(j + 1) * 512],
                                    r(kT[pt][sl:sl + 64, mt * P:(mt + 1) * P]),
                                    r(qT[pt][sl:sl + 64, nt * 512:(nt + 1) * 512]),
                                    start=True, stop=True)
                            sc_sb = p_sc.tile([P, 512], BF16, tag="scsb", name=f"scsb_{h}_{nt}_{gi}")
                            masked = (grp[-1] * P + P - 1) >= nt * 512
                            if masked:
                                for j, mt in enumerate(grp):
                                    off = mt * P - nt * 512   # 0/128/256/384
                                    s0 = j * 512
                                    if off > 0:
                                        nc.gpsimd.memset(sc_sb[:, s0:s0 + off], 0.0)
                                    # diagonal block: mask-multiply (cmask diag
                                    # of section i=off//128 is at abs col
                                    # i*512 + off)
                                    ci = (off // 128) * 512 + off
                                    nc.vector.tensor_mul(
                                        sc_sb[:, s0 + off:s0 + off + P],
                                        ps4[:, s0 + off:s0 + off + P],
                                        cmask[:, ci:ci + P])
                                    if off + P < 512:
                                        nc.scalar.copy(
                                            sc_sb[:, s0 + off + P:s0 + 512],
                                            ps4[:, s0 + off + P:s0 + 512])
                            else:
                                nc.scalar.copy(sc_sb, ps4)
                            for j, mt in enumerate(grp):
                                nc.tensor.matmul(
                                    prt[sl:sl + 64, :],
                                    r(V[mt][:, h * DH:(h + 1) * DH]),
                                    r(sc_sb[:, j * 512:(j + 1) * 512]),
                                    start=(mm_i == 0), stop=(mm_i == n_head_mm - 1),
                                    tile_position=(0, sl), skip_group_check=True)
                                mm_i += 1
                    nc.vector.tensor_copy(retT[pt][:, nt * 512:(nt + 1) * 512], prt)

            if tap == "ret":
                dump_fm(retT)
                return
            # GroupNorm (feature-major, stats over 64 partitions per head).
            # Small tiles: gnA (DVE: mu rows0-1, var rows2-3), gnB (DVE mu^2
            # rows0-1 / ACT sd rows2-3), gnC (DVE rstd rows0-1, feeds PE).
            for pt in range(NF):
                for nt in range(NH2):
                    rsl = retT[pt][:, nt * 512:(nt + 1) * 512]
                    sqt = p_sq.tile([P, 512], BF16, tag="gnsq", name=f"gnsq_{pt}_{nt}", bufs=2)
                    nc.gpsimd.tensor_mul(sqt, rsl, rsl)
                    pstat = pg_d.tile([P, 512], F32, tag="pgd", name=f"gst_{pt}_{nt}")
                    nc.tensor.matmul(pstat[0:2, :], r(gn_ones), r(rsl), start=True, stop=True)
                    pstat2 = pg_d.tile([P, 512], F32, tag="pgd", name=f"gst2_{pt}_{nt}")
                    nc.tensor.matmul(pstat2[0:2, :], r(gn_ones), r(sqt), start=True, stop=True)
                    mu = p_sm.tile([2, 512], BF16, tag="sm", name=f"mu_{pt}_{nt}")
                    mu2 = p_sm.tile([2, 512], F32, tag="sm", name=f"mu2_{pt}_{nt}")
                    var = p_sm.tile([2, 512], F32, tag="sm", name=f"var_{pt}_{nt}")
                    sd = p_sm.tile([2, 512], F32, tag="sm", name=f"sd_{pt}_{nt}")
                    rstd = p_sm.tile([2, 512], BF16, tag="sm", name=f"rstd_{pt}_{nt}")
                    nc.vector.tensor_copy(mu, pstat[0:2, :])
                    nc.vector.tensor_mul(mu2, mu, mu)
                    nc.vector.tensor_sub(var, pstat2[0:2, :], mu2)
                    nc.scalar.activation(sd, var, AF.Sqrt, bias=eps_gn[0:2, :])
                    with nc.allow_low_precision(reason="rstd feeds bf16 broadcast matmul"):
                        nc.vector.reciprocal(rstd, sd)
                    muBp = pg_d.tile([P, 512], F32, tag="pgd", name=f"muBp_{pt}_{nt}")
                    nc.tensor.matmul(muBp, r(gn_bcast), r(mu), start=True, stop=True)
                    rsBp = pg_d.tile([P, 512], F32, tag="pgd", name=f"rsBp_{pt}_{nt}")
                    nc.tensor.matmul(rsBp, r(gn_bcast), r(rstd), start=True, stop=True)
                    nc.vector.tensor_sub(rsl, rsl, muBp)
                    nc.vector.tensor_mul(rsl, rsl, rsBp)
                    nc.scalar.activation(rsl, rsl, AF.Identity,
                                         bias=gb_pp[:, pt:pt + 1], scale=gs_pp[:, pt:pt + 1])

            # gate: g^T = silu(Wg^T @ qsrcT); silu evac on ACT from pg_a
            def evac_g(dst, ps, m, nh):
                nc.scalar.activation(dst, ps, AF.Silu)

            gT = gemm_fm(wg_t, qsrcT, evac_g, out_tag="qT", psum_pool=pg_a)
            # gated = swish(g) * retGN, written into retT (PE reads retT for wo)
            for m in range(NF):
                nc.gpsimd.tensor_mul(retT[m], gT[m], retT[m])

            def evac_o(dst, ps, m, nh):
                nc.scalar.copy(dst, ps)

            gemm_fm(wo_t, retT, evac_o, out_tiles=out_tiles, psum_pool=pg_a)

        # feature-major RMSNorm: out = (a + b) * rsqrt(mean_f((a+b)^2) + eps) * ln
        def rms_fm(aT, bT, ln_pp, out_tag=None, out_tiles=None, out_dt=BF16):
            res = bT
            for k in range(NF):
                nc.vector.tensor_add(res[k], aT[k], bT[k])
            outs = out_tiles
            if outs is None:
                outs = [p_act.tile([P, S], out_dt, tag=f"{out_tag}{k}", name=f"{out_tag}{k}")
                        for k in range(NF)]
            for nh in range(NH2):
                pstat = pg_d.tile([P, 512], F32, tag="pgd", name=f"rst_{nh}")
                for k in range(NF):
                    sqt = p_sq.tile([P, 512], BF16, tag="sq", name=f"rsq_{nh}_{k}")
                    rs = res[k][:, nh * 512:(nh + 1) * 512]
                    nc.vector.tensor_mul(sqt, rs, rs)
                    nc.tensor.matmul(pstat[0:2, :], r(rms_ones), r(sqt),
                                     start=(k == 0), stop=(k == NF - 1))
                r0 = p_sm.tile([2, 512], F32, tag="sm", name=f"r0_{nh}")
                nc.vector.tensor_copy(r0[0:1, :], pstat[0:1, :])
                rA = p_sm.tile([2, 512], F32, tag="sm", name=f"rA_{nh}")
                nc.scalar.activation(rA[0:1, :], r0[0:1, :], AF.Sqrt,
                                     bias=eps_rms[0:1, :], scale=1.0 / E)
                rB = p_sm.tile([2, 512], BF16, tag="sm", name=f"rB_{nh}")
                nc.vector.memset(rB, 0.0)
                with nc.allow_low_precision(reason="rstd feeds bf16 broadcast matmul"):
                    nc.vector.reciprocal(rB[0:1, :], rA[0:1, :])
                rsB = pg_d.tile([P, 512], F32, tag="pgd", name=f"rmsB_{nh}")
                nc.tensor.matmul(rsB, r(rms_bcast), r(rB[0:2, :]), start=True, stop=True)
                for k in range(NF):
                    osl = outs[k][:, nh * 512:(nh + 1) * 512]
                    nc.vector.tensor_mul(osl, res[k][:, nh * 512:(nh + 1) * 512], rsB)
                    nc.gpsimd.tensor_scalar_mul(osl, osl, ln_pp[:, k:k + 1])
            return outs

        # ======== the block ========
        import os
        tap = os.environ.get("KTAP", "")

        def dump_fm(tiles, n_tiles=NF, width=S):
            # write feature-major tiles [128, width] into out rows sequentially
            for k in range(n_tiles):
                t32 = p_ld.tile([P, S], F32, tag="dump", name=f"dmp_{k}", bufs=2)
                nc.vector.tensor_copy(t32[:, :width], tiles[k][:, :width])
                rows = width // E
                for rr in range(rows):
                    nc.sync.dma_start(
                        out=out_h[(k * rows + rr) * P:(k * rows + rr + 1) * P, :],
                        in_=t32[:, rr * E:(rr + 1) * E])
        wq1 = load_w("wqc1"); wk1 = load_w("wkc1"); wv1 = load_w("wvc1")
        wg1 = load_w("wg1"); wo1 = load_w("wo1")
        msr1T = [p_act.tile([P, S], F32, tag=f"msrT{m}", name=f"msr1T{m}") for m in range(NF)]
        if tap == "xT":
            dump_fm(xT)
            return
        msr(xT, xT, wq1, wk1, wv1, wg1, wo1, ppv["gs1_pp"], ppv["gb1_pp"], msr1T,
            tap=tap, dump_fm=dump_fm)
        if tap:
            if tap == "msr1":
                dump_fm(msr1T)
            if tap in ("msr1", "qT", "kT", "V", "ret"):
                return
        x1T = rms_fm(xT, msr1T, ppv["ln1_pp"], out_tag="x1T")
        if tap == "x1":
            dump_fm(x1T)
            return

        wq2 = load_w("wqc2"); wk2 = load_w("wkc2"); wv2 = load_w("wvc2")
        wg2 = load_w("wg2"); wo2 = load_w("wo2")
        msr2T = [p_act.tile([P, S], F32, tag=f"msrT{m}", name=f"msr2T{m}") for m in range(NF)]
        msr(obsT, x1T, wq2, wk2, wv2, wg2, wo2, ppv["gs2_pp"], ppv["gb2_pp"], msr2T)
        x2T = rms_fm(obsT, msr2T, ppv["ln2_pp"], out_tag="xT")  # reuse xT slots

        def load_w_tags(nm, tags):
            tiles = []
            for k in range(NF):
                wt = p_act.tile([P, E], BF16, tag=tags[k], name=f"{nm}_{k}")
                nc.sync.dma_start(out=wt, in_=d[nm][k * P:(k + 1) * P, :])
                tiles.append(wt)
            return tiles

        wfg = load_w_tags("ffn_w_gate", [f"V{i}" for i in range(4)])
        wfl = load_w("ffn_w_lin")
        wfo = load_w("ffn_w_out")

        def evac_silu(dst, ps, m, nh):
            nc.scalar.activation(dst, ps, AF.Silu)

        def evac_cp_d(dst, ps, m, nh):
            nc.scalar.copy(dst, ps)

        fgT = gemm_fm(wfg, x2T, evac_silu, out_tag="qT", psum_pool=pg_a)
        flT = gemm_fm(wfl, x2T, evac_cp_d, out_tag="kT", psum_pool=pg_a)
        for m in range(NF):
            nc.gpsimd.tensor_mul(flT[m], fgT[m], flT[m])
        ffnT = gemm_fm(wfo, flT, evac_cp_d, out_tag="x1T", out_dt=F32, psum_pool=pg_a)
        x3T = rms_fm(x2T, ffnT, ppv["ln3_pp"], out_tag="msrT", out_dt=F32)

        # ---- output transpose: x3T [E,S] -> out [S,E] ----
        for sidx in range(NS):
            ps = pg_d.tile([P, E], F32, tag="pgd", name=f"ot_{sidx}")
            for k in range(NF):
                nc.tensor.matmul(ps[:, k * P:(k + 1) * P],
                                 x3T[k][:, sidx * P:(sidx + 1) * P], ident,
                                 is_transpose=True,
                                 start=(k == 0), stop=(k == NF - 1))
            ot = p_ld.tile([P, E], F32, tag="ot", name=f"ot_{sidx}", bufs=1)
            nc.vector.tensor_copy(ot, ps)
            nc.sync.dma_start(out=out_h[sidx * P:(sidx + 1) * P, :], in_=ot)


_prog_cache = None


def _get_program():
    global _prog_cache
    if _prog_cache is None:
        _prog_cache = _build_program()
    return _prog_cache


def kernel(**inputs):
    inputs = {k: np.asarray(v) for k, v in inputs.items()}
    consts = _build_consts(inputs)
    pr = _get_program()
    shared = dict(consts)
    x = np.ascontiguousarray(inputs["x"], dtype=np.float32)
    obs = np.ascontiguousarray(inputs["obs_rep"], dtype=np.float32)
    in_maps = []
    for b in range(N_CORES):
        m = dict(shared)
        m["x"] = np.ascontiguousarray(x[b])
        m["obs"] = np.ascontiguousarray(obs[b])
        in_maps.append(m)
    res = run_bass_kernel_spmd(pr.nc, in_maps, core_ids=list(range(N_CORES)))
    return np.stack([res.results[b]["out"] for b in range(N_CORES)], axis=0)


# revision 57
# speedup vs baseline: 1.1545x; 1.0073x over previous
"""Trainium2 Bass kernel for nn_DecodeBlock (RetNet-style decoder block).

Sharding: data-parallel over batch (B=8) across the 8 NeuronCores; each core
computes the full block for one batch element. No collectives.

Algorithm notes (per core, feature-major "transposed" dataflow):
  - All activations are kept feature-major: X^T [E=512(4 part-tiles), S=1024].
  - Retention decay D[h,n,m] = kappa_h^(n-m) (causal) is applied via global
    row/col scaling: qs^T = q^T * kappa^n, ks^T = k^T * kappa^-m, then a 0/1
    causal mask on diagonal blocks only (exact in fp32: kappa^-1023 <= 1.3e14).
  - scoresT[m,n] tiles come straight from PE with m on partitions; ret^T is
    accumulated per head with V (seq-major) as the stationary operand.
  - GroupNorm/RMSNorm stats are computed with ones-matmuls over partitions
    (PE) and broadcast back with gpsimd partition_broadcast.
  - Matmuls run as float32r (TF32-like) at free-dim 512 -> 1 cycle/row.
"""

import numpy as np

import concourse.bass as bass
import concourse.mybir as mybir
import concourse.tile as tile
from concourse.bass_utils import run_bass_kernel_spmd

F32 = mybir.dt.float32
BF16 = mybir.dt.bfloat16
AF = mybir.ActivationFunctionType

E, H, B, S = 512, 8, 8, 1024
DH = E // H          # 64
P = 128
NF = E // P          # 4 feature tiles
NS = S // P          # 8 seq tiles
NH2 = S // 512       # 2 n-halves

N_CORES = 8


def _kappas():
    k = 1.0 - np.exp(np.linspace(np.log(1.0 / 32.0), np.log(1.0 / 512.0), H))
    return k.astype(np.float64)


def r(ap):
    return ap


def _build_consts(inputs):
    """Host-side constant tensors shared by all cores."""
    import ml_dtypes
    bf16 = ml_dtypes.bfloat16
    kap = _kappas()
    n = np.arange(S, dtype=np.float64)
    kq = np.empty((E, S), np.float64)
    kk = np.empty((E, S), np.float64)
    for h in range(H):
        kq[h * DH:(h + 1) * DH, :] = (kap[h] ** n)[None, :]
        kk[h * DH:(h + 1) * DH, :] = (kap[h] ** (-n))[None, :]
    kqmap = np.ascontiguousarray(kq.astype(bf16))
    kkmap = np.ascontiguousarray(kk.astype(bf16))
    # causal mask for a [128, 4*512] psum group: section i covers m-block
    # offset 128*i vs n-block base: keep if j >= p + 128*i
    cmask = np.zeros((P, 4 * 512), np.float32)  # cast to bf16 below
    jj = np.arange(512)
    for i in range(4):
        cmask[:, i * 512:(i + 1) * 512] = (jj[None, :] >= (np.arange(P)[:, None] + 128 * i))
    cmask = cmask.astype(bf16)
    gn_ones = np.zeros((P, 2), bf16)
    gn_ones[:64, 0] = 1.0 / DH
    gn_ones[64:, 1] = 1.0 / DH
    gn_bcast = np.zeros((2, P), bf16)
    gn_bcast[0, :64] = 1.0
    gn_bcast[1, 64:] = 1.0
    rms_ones = np.zeros((P, 2), bf16)
    rms_ones[:, 0] = 1.0
    rms_bcast = np.zeros((2, P), bf16)
    rms_bcast[0, :] = 1.0
    ident = np.eye(P, dtype=np.float32)

    def pp(v):  # [512] -> [128, 4] per-partition layout
        return np.ascontiguousarray(np.asarray(v, np.float32).reshape(NF, P).T)

    consts = {
        "kqmap": kqmap, "kkmap": kkmap, "cmask": cmask,
        "gn_ones": gn_ones, "gn_bcast": gn_bcast, "rms_ones": rms_ones,
        "rms_bcast": rms_bcast, "ident": ident,
        "gs1_pp": pp(inputs["gs1"]), "gb1_pp": pp(inputs["gb1"]),
        "gs2_pp": pp(inputs["gs2"]), "gb2_pp": pp(inputs["gb2"]),
        "ln1_pp": pp(inputs["ln1_s"]), "ln2_pp": pp(inputs["ln2_s"]),
        "ln3_pp": pp(inputs["ln3_s"]),
        "rl1": np.vstack([np.asarray(inputs["ln1_s"], np.float32),
                          np.zeros(E, np.float32)]).astype(bf16),
        "rl2": np.vstack([np.asarray(inputs["ln2_s"], np.float32),
                          np.zeros(E, np.float32)]).astype(bf16),
        "rl3": np.vstack([np.asarray(inputs["ln3_s"], np.float32),
                          np.zeros(E, np.float32)]).astype(bf16),
    }
    for nm in ("wq", "wk", "wv"):
        for i in (1, 2):
            w = np.asarray(inputs[f"{nm}{i}"], np.float32)      # [H, E, DH]
            consts[f"{nm}c{i}"] = np.ascontiguousarray(
                w.transpose(1, 0, 2).reshape(E, E).astype(bf16))
    for nm in ("wg1", "wo1", "wg2", "wo2", "ffn_w_gate", "ffn_w_lin", "ffn_w_out"):
        consts[nm] = np.ascontiguousarray(np.asarray(inputs[nm], np.float32).astype(bf16))
    return consts


class _Prog:
    pass


def _build_program():
    nc = bass.Bass()
    pr = _Prog()
    pr.nc = nc
    d = {}
    d["x"] = nc.dram_tensor("x", [S, E], F32, kind="ExternalInput")
    d["obs"] = nc.dram_tensor("obs", [S, E], F32, kind="ExternalInput")
    for nm in ("wqc1", "wkc1", "wvc1", "wqc2", "wkc2", "wvc2",
               "wg1", "wo1", "wg2", "wo2",
               "ffn_w_gate", "ffn_w_lin", "ffn_w_out"):
        d[nm] = nc.dram_tensor(nm, [E, E], BF16, kind="ExternalInput")
    d["cmask"] = nc.dram_tensor("cmask", [P, 4 * 512], BF16, kind="ExternalInput")
    d["gn_ones"] = nc.dram_tensor("gn_ones", [P, 2], BF16, kind="ExternalInput")
    d["gn_bcast"] = nc.dram_tensor("gn_bcast", [2, P], BF16, kind="ExternalInput")
    d["rms_ones"] = nc.dram_tensor("rms_ones", [P, 2], BF16, kind="ExternalInput")
    d["rms_bcast"] = nc.dram_tensor("rms_bcast", [2, P], BF16, kind="ExternalInput")
    d["ident"] = nc.dram_tensor("ident", [P, P], F32, kind="ExternalInput")
    for nm in ("gs1_pp", "gb1_pp", "gs2_pp", "gb2_pp", "ln1_pp", "ln2_pp", "ln3_pp"):
        d[nm] = nc.dram_tensor(nm, [P, NF], F32, kind="ExternalInput")
    for nm in ("rl1", "rl2", "rl3"):
        d[nm] = nc.dram_tensor(nm, [2, E], BF16, kind="ExternalInput")
    d["kqmap"] = nc.dram_tensor("kqmap", [E, S], BF16, kind="ExternalInput")
    d["kkmap"] = nc.dram_tensor("kkmap", [E, S], BF16, kind="ExternalInput")
    out_h = nc.dram_tensor("out", [S, E], F32, kind="ExternalOutput")

    with tile.TileContext(nc) as tc:
        _emit(nc, tc, d, out_h)
    _strip_self_waits(nc)
    _legalize_wait_counts(nc)
    return pr


_ENGINE_PROC = {
    "PE": "PE", "DVE": "DVE", "Activation": "Activation",
    "Pool": "Pool", "SP": "SP",
}


def _strip_self_waits(nc):
    """Remove same-engine sem waits on engine compute instructions.

    Engines execute their FIFO in order (DVE/ACT drain between ops; PE only
    reorders LDWEIGHTS pull-ahead, and PE never writes SBUF), so a wait on
    the instruction's own engine semaphore is redundant — and walrus only
    allows 2 sync waits per instruction."""
    import concourse.mybir as mb
    for f in nc.m.functions:
        for blk in f.blocks:
            for inst in blk.instructions:
                si = getattr(inst, "sync_info", None)
                if si is None or not si.on_wait:
                    continue
                tname = type(inst).__name__
                if tname in ("InstDMACopy", "InstDrain", "InstEventSemaphore",
                             "InstTriggerDma"):
                    continue
                eng = getattr(inst, "engine", None)
                eng_name = getattr(eng, "name", str(eng))
                pref = {"PE": "PE_", "DVE": "DVE_", "Activation": "Activation_",
                        "Pool": "Pool_", "SP": "SP_"}.get(eng_name)
                if not pref:
                    continue
                kept = [w for w in si.on_wait if not str(w.ant_name).startswith(pref)]
                if len(kept) != len(si.on_wait):
                    si.on_wait = kept


def _bc(row_ap, n_part):
    """Partition-broadcast read AP: replicate a single-partition row across
    n_part partitions (partition-step-0 source, for DMA)."""
    return bass.AP(tensor=row_ap.tensor, offset=row_ap.offset,
                   ap=[[0, n_part]] + [list(p) for p in row_ap.ap[1:]])


_MAX_WAITS = 1
_WAIT_BUDGET = {"InstActivation": 1, "InstDrain": 0}


def _legalize_wait_counts(nc):
    """walrus allows at most 2 sync waits per lowered instruction. Move any
    excess waits onto injected same-engine sequencer NOPs placed immediately
    before the offending instruction (program order on the engine's stream
    gates the instruction behind the NOP's waits)."""
    import bass_rust
    import concourse.mybir as mb
    uid = [0]
    for f in nc.m.functions:
        for blk in f.blocks:
            insts = list(blk.instructions)
            out = []
            changed = False
            for inst in insts:
                si = getattr(inst, "sync_info", None)
                waits = list(si.on_wait) if si and si.on_wait else []
                plain = [w for w in waits if w.sync_type == "semaphore"]
                other = [w for w in waits if w.sync_type != "semaphore"]
                cap = _WAIT_BUDGET.get(type(inst).__name__, _MAX_WAITS)
                if len(plain) + len(other) > cap and len(plain) > 0:
                    budget = max(0, cap - len(other))
                    keep, excess = plain[:budget], plain[budget:]
                    while excess:
                        chunk, excess = excess[:1], excess[1:]
                        nop = bass_rust.InstNoOp(name=f"wnop-{uid[0]}", ins=[], outs=[])
                        uid[0] += 1
                        nop.engine = inst.engine
                        nop.sync_info = mb.SyncInfo(on_wait=chunk, on_update=[])
                        out.append(nop)
                    si.on_wait = other + keep
                    changed = True
                out.append(inst)
            if changed:
                blk.instructions = out


def _emit(nc, tc, d, out_h):
    from contextlib import ExitStack
    ctx = ExitStack()
    with ctx:
        # Pools. Wait-limit discipline: every instruction may carry at most 2
        # sync waits after walrus lowering, so each tile has a single writer
        # engine and PSUM pools are split by evacuating engine (pg_d -> DVE,
        # pg_a -> ACT).
        p_const = ctx.enter_context(tc.tile_pool(name="const", bufs=1))
        p_act = ctx.enter_context(tc.tile_pool(name="act", bufs=1))
        p_w = ctx.enter_context(tc.tile_pool(name="w", bufs=12))
        p_map = ctx.enter_context(tc.tile_pool(name="map", bufs=8))
        p_sc = ctx.enter_context(tc.tile_pool(name="sc", bufs=8))
        p_sq = ctx.enter_context(tc.tile_pool(name="sq", bufs=2))
        p_sm = ctx.enter_context(tc.tile_pool(name="sm", bufs=8))
        # note: ld pool shares output tiles
        p_ld = ctx.enter_context(tc.tile_pool(name="ld", bufs=4))
        pg_d = ctx.enter_context(tc.tile_pool(name="pgd", bufs=2, space="PSUM"))
        pg_a = ctx.enter_context(tc.tile_pool(name="pga", bufs=2, space="PSUM"))
        psc = ctx.enter_context(tc.tile_pool(name="psc", bufs=3, space="PSUM"))
        pret = ctx.enter_context(tc.tile_pool(name="pret", bufs=1, space="PSUM"))

        # ---- constants ----
        cmask = p_const.tile([P, 4 * 512], BF16)
        nc.sync.dma_start(out=cmask, in_=d["cmask"][:, :])
        gn_ones = p_const.tile([P, 2], BF16)
        nc.sync.dma_start(out=gn_ones, in_=d["gn_ones"][:, :])
        gn_bcast = p_const.tile([2, P], BF16)
        nc.sync.dma_start(out=gn_bcast, in_=d["gn_bcast"][:, :])
        rms_ones = p_const.tile([P, 2], BF16)
        nc.sync.dma_start(out=rms_ones, in_=d["rms_ones"][:, :])
        rms_bcast = p_const.tile([2, P], BF16)
        nc.sync.dma_start(out=rms_bcast, in_=d["rms_bcast"][:, :])
        ident = p_const.tile([P, P], F32)
        nc.sync.dma_start(out=ident, in_=d["ident"][:, :])
        ppv = {}
        for nm in ("gs1_pp", "gb1_pp", "gs2_pp", "gb2_pp", "ln1_pp", "ln2_pp", "ln3_pp"):
            t = p_const.tile([P, NF], F32, name=nm)
            nc.sync.dma_start(out=t, in_=d[nm][:, :])
            ppv[nm] = t
        rlv = {}
        for nm in ("rl1", "rl2", "rl3"):
            t = p_const.tile([2, E], BF16, name=nm)
            nc.sync.dma_start(out=t, in_=d[nm][:, :])
            rlv[nm] = t
        eps_gn = p_const.tile([P, 1], F32)
        nc.vector.memset(eps_gn, 1e-5)
        eps_rms = p_const.tile([P, 1], F32)
        nc.vector.memset(eps_rms, 1e-6)
        zero_d = p_const.tile([P, 1], F32)
        nc.vector.memset(zero_d, 0.0)
        # DVE observers for HWDGE const queues (keeps later DVE ops <=2 waits)
        wuv = p_const.tile([P, 1], F32)
        for cn in list(ppv.values()) + [cmask]:
            nc.vector.tensor_copy(wuv, cn[:, 0:1])

        # Warmup matmuls so PE observes each PE-read constant's DMA queue sem
        # early (keeps later matmuls at <=2 waits).
        wu = pg_d.tile([P, P], F32, tag="pgd", name="wu")
        nc.tensor.matmul(wu[0:2, 0:P], gn_ones, cmask[:, 0:P], start=True, stop=True)
        nc.tensor.matmul(wu[0:P, 0:P], gn_bcast, gn_bcast, start=False, stop=True,
                         skip_group_check=True)
        nc.tensor.matmul(wu[0:2, 0:P], rms_ones, cmask[:, 0:P], start=False, stop=True,
                         skip_group_check=True)
        nc.tensor.matmul(wu[0:P, 0:P], rms_bcast, rms_bcast, start=False, stop=True,
                         skip_group_check=True)

        def load_w(nm, tag="w"):
            tiles = []
            for k in range(NF):
                wt = p_w.tile([P, E], BF16, tag=tag, name=f"{nm}_{k}")
                nc.sync.dma_start(out=wt, in_=d[nm][k * P:(k + 1) * P, :])
                tiles.append(wt)
            return tiles

        # ---- phase 0: load + transpose x, obs -> xT, obsT (evac: DVE only) ----
        def transpose_in(src_h, out_tag):
            outT = []
            for k in range(NF):
                t = p_act.tile([P, S], BF16, tag=f"{out_tag}{k}", name=f"{out_tag}{k}")
                outT.append(t)
            for sidx in range(NS):
                s_sb = p_ld.tile([P, E], F32, tag="ld", name=f"ld_{sidx}")
                nc.sync.dma_start(out=s_sb, in_=src_h[sidx * P:(sidx + 1) * P, :])
                ps = pg_d.tile([P, E], F32, tag="pgd", name=f"tp_{sidx}")
                for k in range(NF):
                    nc.tensor.matmul(ps[:, k * P:(k + 1) * P], s_sb[:, k * P:(k + 1) * P],
                                     ident, is_transpose=True,
                                     start=(k == 0), stop=(k == NF - 1))
                for k in range(NF):
                    nc.scalar.copy(outT[k][:, sidx * P:(sidx + 1) * P],
                                   ps[:, k * P:(k + 1) * P])
            return outT

        xT = transpose_in(d["x"], "xT")
        obsT = transpose_in(d["obs"], "obsT")

        # ---- helper: [E,E] gemm, out feature-major: outT = W^T @ srcT ----
        def gemm_fm(w_tiles, srcT, evac, out_tag=None, out_tiles=None, out_dt=BF16,
                    psum_pool=None):
            pool = psum_pool or pg_d
            outs = out_tiles
            if outs is None:
                outs = [p_act.tile([P, S], out_dt, tag=f"{out_tag}{m}", name=f"{out_tag}{m}")
                        for m in range(NF)]
            for m in range(NF):
                for nh in range(NH2):
                    ps = pool.tile([P, 512], F32, tag=pool.name, name=f"g_{m}_{nh}")
                    for k in range(NF):
                        nc.tensor.matmul(
                            ps, r(w_tiles[k][:, m * P:(m + 1) * P]),
                            r(srcT[k][:, nh * 512:(nh + 1) * 512]),
                            start=(k == 0), stop=(k == NF - 1))
                    evac(outs[m][:, nh * 512:(nh + 1) * 512], ps, m, nh)
            return outs

        def msr(qsrcT, kvsrcT, wq_t, wk_t, wv_t, wg_t, wo_t, gs_pp, gb_pp, out_tiles,
                tap="", dump_fm=None):
            # q^T / k^T: plain DVE evac, then in-place decay-map multiply
            # (maps precomputed on host, streamed from HBM).
            def mk_evac_map(map_h):
                def evac(dst, ps, m, nh):
                    mt = p_map.tile([P, 512], BF16, tag="map", name=f"map_{m}")
                    nc.sync.dma_start(
                        out=mt, in_=map_h[m * P:(m + 1) * P, nh * 512:(nh + 1) * 512])
                    nc.vector.tensor_mul(dst, ps, mt)
                return evac

            qT = gemm_fm(wq_t, qsrcT, mk_evac_map(d["kqmap"]), out_tag="qT")
            if tap == "qT":
                dump_fm(qT)
                return
            kT = gemm_fm(wk_t, kvsrcT, mk_evac_map(d["kkmap"]), out_tag="kT")
            if tap == "kT":
                dump_fm(kT)
                return

            # V seq-major: V[st] [128, 512(all heads)]
            V = []
            for st in range(NS):
                ps = pg_d.tile([P, 512], F32, tag="pgd", name=f"v_{st}")
                for k in range(NF):
                    nc.tensor.matmul(ps, r(kvsrcT[k][:, st * P:(st + 1) * P]), r(wv_t[k]),
                                     start=(k == 0), stop=(k == NF - 1))
                vt = p_act.tile([P, 512], BF16, tag=f"V{st}", name=f"V{st}")
                nc.scalar.copy(vt, ps)
                V.append(vt)
            if tap == "V":
                dump_fm(V, n_tiles=NS, width=E)
                return

            # scores + ret; two heads (one pair tile) share a ret psum bank:
            # even head -> rows 0:64, odd head -> rows 64:128 (col group).
            retT = [p_act.tile([P, S], BF16, tag=f"retT{pt}", name=f"retT{pt}") for pt in range(NF)]
            for pt in range(NF):
                for nt in range(NH2):
                    prt = pret.tile([P, 512], F32, tag="pret", name=f"pret_{pt}_{nt}")
                    groups = ([[0], [1], [2], [3]] if nt == 0
                              else [[0], [1], [2], [3], [4], [5], [6], [7]])
                    n_head_mm = sum(len(g) for g in groups)
                    for hh in range(2):      # head within pair
                        mm_i = 0
                        h = pt * 2 + hh
                        sl = hh * 64
                        for gi, grp in enumerate(groups):
                            ps4 = psc.tile([P, 512], F32, tag="psc", name=f"sc_{h}_{nt}_{gi}")
                            for j, mt in enumerate(grp):
                                # each j targets its own PSUM bank -> own group
                                nc.tensor.matmul(
                                    ps4[:, j * 512:(j + 1) * 512],
                                    r(kT[pt][sl:sl + 64, mt * P:(mt + 1) * P]),
                                    r(qT[pt][sl:sl + 64, nt * 512:(nt + 1) * 512]),
                                    start=True, stop=True)
                            sc_sb = p_sc.tile([P, 512], BF16, tag="scsb", name=f"scsb_{h}_{nt}_{gi}")
                            masked = (grp[-1] * P + P - 1) >= nt * 512
                            if masked:
                                for j, mt in enumerate(grp):
                                    off = mt * P - nt * 512   # 0/128/256/384
                                    s0 = j * 512
                                    if off > 0:
                                        nc.gpsimd.memset(sc_sb[:, s0:s0 + off], 0.0)
                                    # diagonal block: mask-multiply (cmask diag
                                    # of section i=off//128 is at abs col
                                    # i*512 + off)
                                    ci = (off // 128) * 512 + off
                                    nc.vector.tensor_mul(
                                        sc_sb[:, s0 + off:s0 + off + P],
                                        ps4[:, s0 + off:s0 + off + P],
                                        cmask[:, ci:ci + P])
                                    if off + P < 512:
                                        nc.scalar.copy(
                                            sc_sb[:, s0 + off + P:s0 + 512],
                                            ps4[:, s0 + off + P:s0 + 512])
                            else:
                                nc.scalar.copy(sc_sb, ps4)
                            for j, mt in enumerate(grp):
                                nc.tensor.matmul(
                                    prt[sl:sl + 64, :],
                                    r(V[mt][:, h * DH:(h + 1) * DH]),
                                    r(sc_sb[:, j * 512:(j + 1) * 512]),
                                    start=(mm_i == 0), stop=(mm_i == n_head_mm - 1),
                                    tile_position=(0, sl), skip_group_check=True)
                                mm_i += 1
                    nc.vector.tensor_copy(retT[pt][:, nt * 512:(nt + 1) * 512], prt)

            if tap == "ret":
                dump_fm(retT)
                return
            # GroupNorm (feature-major, stats over 64 partitions per head).
            # Small tiles: gnA (DVE: mu rows0-1, var rows2-3), gnB (DVE mu^2
            # rows0-1 / ACT sd rows2-3), gnC (DVE rstd rows0-1, feeds PE).
            for pt in range(NF):
                for nt in range(NH2):
                    rsl = retT[pt][:, nt * 512:(nt + 1) * 512]
                    sqt = p_sq.tile([P, 512], BF16, tag="gnsq", name=f"gnsq_{pt}_{nt}", bufs=2)
                    nc.gpsimd.tensor_mul(sqt, rsl, rsl)
                    pstat = pg_d.tile([P, 512], F32, tag="pgd", name=f"gst_{pt}_{nt}")
                    nc.tensor.matmul(pstat[0:2, :], r(gn_ones), r(rsl), start=True, stop=True)
                    pstat2 = pg_d.tile([P, 512], F32, tag="pgd", name=f"gst2_{pt}_{nt}")
                    nc.tensor.matmul(pstat2[0:2, :], r(gn_ones), r(sqt), start=True, stop=True)
                    mu = p_sm.tile([2, 512], BF16, tag="sm", name=f"mu_{pt}_{nt}")
                    mu2 = p_sm.tile([2, 512], F32, tag="sm", name=f"mu2_{pt}_{nt}")
                    var = p_sm.tile([2, 512], F32, tag="sm", name=f"var_{pt}_{nt}")
                    sd = p_sm.tile([2, 512], F32, tag="sm", name=f"sd_{pt}_{nt}")
                    rstd = p_sm.tile([2, 512], BF16, tag="sm", name=f"rstd_{pt}_{nt}")
                    nc.vector.tensor_copy(mu, pstat[0:2, :])
                    nc.vector.tensor_mul(mu2, mu, mu)
                    nc.vector.tensor_sub(var, pstat2[0:2, :], mu2)
                    nc.scalar.activation(sd, var, AF.Sqrt, bias=eps_gn[0:2, :])
                    with nc.allow_low_precision(reason="rstd feeds bf16 broadcast matmul"):
                        nc.vector.reciprocal(rstd, sd)
                    muBp = pg_d.tile([P, 512], F32, tag="pgd", name=f"muBp_{pt}_{nt}")
                    nc.tensor.matmul(muBp, r(gn_bcast), r(mu), start=True, stop=True)
                    rsBp = pg_d.tile([P, 512], F32, tag="pgd", name=f"rsBp_{pt}_{nt}")
                    nc.tensor.matmul(rsBp, r(gn_bcast), r(rstd), start=True, stop=True)
                    nc.vector.tensor_sub(rsl, rsl, muBp)
                    nc.vector.tensor_mul(rsl, rsl, rsBp)
                    nc.scalar.activation(rsl, rsl, AF.Identity,
                                         bias=gb_pp[:, pt:pt + 1], scale=gs_pp[:, pt:pt + 1])

            # gate: g^T = silu(Wg^T @ qsrcT); silu evac on ACT from pg_a
            def evac_g(dst, ps, m, nh):
                nc.scalar.activation(dst, ps, AF.Silu)

            gT = gemm_fm(wg_t, qsrcT, evac_g, out_tag="qT", psum_pool=pg_a)
            # gated = swish(g) * retGN, written into retT (PE reads retT for wo)
            for m in range(NF):
                nc.gpsimd.tensor_mul(retT[m], gT[m], retT[m])

            def evac_o(dst, ps, m, nh):
                nc.scalar.copy(dst, ps)

            gemm_fm(wo_t, retT, evac_o, out_tiles=out_tiles, psum_pool=pg_a)

        # feature-major RMSNorm: out = (a + b) * rsqrt(mean_f((a+b)^2) + eps) * ln
        def rms_fm(aT, bT, ln_pp, out_tag=None, out_tiles=None, out_dt=BF16):
            res = bT
            for k in range(NF):
                nc.vector.tensor_add(res[k], aT[k], bT[k])
            outs = out_tiles
            if outs is None:
                outs = [p_act.tile([P, S], out_dt, tag=f"{out_tag}{k}", name=f"{out_tag}{k}")
                        for k in range(NF)]
            for nh in range(NH2):
                pstat = pg_d.tile([P, 512], F32, tag="pgd", name=f"rst_{nh}")
                for k in range(NF):
                    sqt = p_sq.tile([P, 512], BF16, tag="sq", name=f"rsq_{nh}_{k}")
                    rs = res[k][:, nh * 512:(nh + 1) * 512]
                    nc.vector.tensor_mul(sqt, rs, rs)
                    nc.tensor.matmul(pstat[0:2, :], r(rms_ones), r(sqt),
                                     start=(k == 0), stop=(k == NF - 1))
                r0 = p_sm.tile([2, 512], F32, tag="sm", name=f"r0_{nh}")
                nc.vector.tensor_copy(r0[0:1, :], pstat[0:1, :])
                rA = p_sm.tile([2, 512], F32, tag="sm", name=f"rA_{nh}")
                nc.scalar.activation(rA[0:1, :], r0[0:1, :], AF.Sqrt,
                                     bias=eps_rms[0:1, :], scale=1.0 / E)
                rB = p_sm.tile([2, 512], BF16, tag="sm", name=f"rB_{nh}")
                nc.vector.memset(rB, 0.0)
                with nc.allow_low_precision(reason="rstd feeds bf16 broadcast matmul"):
                    nc.vector.reciprocal(rB[0:1, :], rA[0:1, :])
                rsB = pg_d.tile([P, 512], F32, tag="pgd", name=f"rmsB_{nh}")
                nc.tensor.matmul(rsB, r(rms_bcast), r(rB[0:2, :]), start=True, stop=True)
                for k in range(NF):
                    osl = outs[k][:, nh * 512:(nh + 1) * 512]
                    nc.vector.tensor_mul(osl, res[k][:, nh * 512:(nh + 1) * 512], rsB)
                    nc.gpsimd.tensor_scalar_mul(osl, osl, ln_pp[:, k:k + 1])
            return outs

        # ======== the block ========
        import os
        tap = os.environ.get("KTAP", "")

        def dump_fm(tiles, n_tiles=NF, width=S):
            # write feature-major tiles [128, width] into out rows sequentially
            for k in range(n_tiles):
                t32 = p_ld.tile([P, S], F32, tag="dump", name=f"dmp_{k}", bufs=2)
                nc.vector.tensor_copy(t32[:, :width], tiles[k][:, :width])
                rows = width // E
                for rr in range(rows):
                    nc.sync.dma_start(
                        out=out_h[(k * rows + rr) * P:(k * rows + rr + 1) * P, :],
                        in_=t32[:, rr * E:(rr + 1) * E])
        wq1 = load_w("wqc1"); wk1 = load_w("wkc1"); wv1 = load_w("wvc1")
        wg1 = load_w("wg1"); wo1 = load_w("wo1")
        msr1T = [p_act.tile([P, S], F32, tag=f"msrT{m}", name=f"msr1T{m}") for m in range(NF)]
        if tap == "xT":
            dump_fm(xT)
            return
        msr(xT, xT, wq1, wk1, wv1, wg1, wo1, ppv["gs1_pp"], ppv["gb1_pp"], msr1T,
            tap=tap, dump_fm=dump_fm)
        if tap:
            if tap == "msr1":
                dump_fm(msr1T)
            if tap in ("msr1", "qT", "kT", "V", "ret"):
                return
        x1T = rms_fm(xT, msr1T, ppv["ln1_pp"], out_tag="x1T")
        if tap == "x1":
            dump_fm(x1T)
            return

        wq2 = load_w("wqc2"); wk2 = load_w("wkc2"); wv2 = load_w("wvc2")
        wg2 = load_w("wg2"); wo2 = load_w("wo2")
        msr2T = [p_act.tile([P, S], F32, tag=f"msrT{m}", name=f"msr2T{m}") for m in range(NF)]
        msr(obsT, x1T, wq2, wk2, wv2, wg2, wo2, ppv["gs2_pp"], ppv["gb2_pp"], msr2T)
        x2T = rms_fm(obsT, msr2T, ppv["ln2_pp"], out_tag="xT")  # reuse xT slots

        def load_w_tags(nm, tags):
            tiles = []
            for k in range(NF):
                wt = p_act.tile([P, E], BF16, tag=tags[k], name=f"{nm}_{k}")
                nc.sync.dma_start(out=wt, in_=d[nm][k * P:(k + 1) * P, :])
                tiles.append(wt)
            return tiles

        wfg = load_w_tags("ffn_w_gate", [f"V{i}" for i in range(4)])
        wfl = load_w("ffn_w_lin")
        wfo = load_w("ffn_w_out")

        def evac_silu(dst, ps, m, nh):
            nc.scalar.activation(dst, ps, AF.Silu)

        def evac_cp_d(dst, ps, m, nh):
            nc.scalar.copy(dst, ps)

        fgT = gemm_fm(wfg, x2T, evac_silu, out_tag="qT", psum_pool=pg_a)
        flT = gemm_fm(wfl, x2T, evac_cp_d, out_tag="kT", psum_pool=pg_a)
        for m in range(NF):
            nc.gpsimd.tensor_mul(flT[m], fgT[m], flT[m])
        ffnT = gemm_fm(wfo, flT, evac_cp_d, out_tag="x1T", out_dt=F32, psum_pool=pg_a)
        x3T = rms_fm(x2T, ffnT, ppv["ln3_pp"], out_tag="msrT", out_dt=F32)

        # ---- output transpose: x3T [E,S] -> out [S,E] ----
        for sidx in range(NS):
            ps = pg_d.tile([P, E], F32, tag="pgd", name=f"ot_{sidx}")
            for k in range(NF):
                nc.tensor.matmul(ps[:, k * P:(k + 1) * P],
                                 x3T[k][:, sidx * P:(sidx + 1) * P], ident,
                                 is_transpose=True,
                                 start=(k == 0), stop=(k == NF - 1))
            ot = p_ld.tile([P, E], F32, tag="ot", name=f"ot_{sidx}", bufs=1)
            nc.vector.tensor_copy(ot, ps)
            nc.sync.dma_start(out=out_h[sidx * P:(sidx + 1) * P, :], in_=ot)


_prog_cache = None


def _get_program():
    global _prog_cache
    if _prog_cache is None:
        _prog_cache = _build_program()
    return _prog_cache


def kernel(**inputs):
    inputs = {k: np.asarray(v) for k, v in inputs.items()}
    consts = _build_consts(inputs)
    pr = _get_program()
    shared = dict(consts)
    x = np.ascontiguousarray(inputs["x"], dtype=np.float32)
    obs = np.ascontiguousarray(inputs["obs_rep"], dtype=np.float32)
    in_maps = []
    for b in range(N_CORES):
        m = dict(shared)
        m["x"] = np.ascontiguousarray(x[b])
        m["obs"] = np.ascontiguousarray(obs[b])
        in_maps.append(m)
    res = run_bass_kernel_spmd(pr.nc, in_maps, core_ids=list(range(N_CORES)))
    return np.stack([res.results[b]["out"] for b in range(N_CORES)], axis=0)
